# revision 16
# baseline (speedup 1.0000x reference)
"""CLIP contrastive loss on 8 Trainium2 NeuronCores (Bass/Tile).

Strategy (data-parallel over image rows, hint's local_loss path):
  - Core c holds image rows [c*1024, (c+1)*1024) and the FULL text matrix.
  - Text rows are rolled by c*1024 on the host so every core's diagonal
    block sits at local column 0 (the compiled program is core-independent).
  - On device, each core computes its 1024 x 8192 logits block in
    128x512 PSUM tiles (4 accumulating K=128 matmuls each), then:
      * ACT: exp(scale*s - shift) PSUM->SBUF, accum_out = per-row sums
      * PE:  ones[128,1]^T @ exp_tile accumulated in PSUM = column sums
      * DVE: tensor_tensor_reduce against scale*I extracts the diagonal
  - Host: combines per-core row/col exp-sums and diagonals in float64:
      lse = shift + log(sum); loss = mean over both directions.

Fixed-shift logsumexp is numerically safe: logits = scale*cos(theta) are
bounded by +-scale, and shift = scale/2 keeps every term that matters in
normal f32 range (terms below exp(-87) are negligible vs the row max).
"""

from contextlib import ExitStack

import numpy as np

import concourse.bass as bass
from concourse import bacc
import concourse.tile as tile
from concourse import mybir
from concourse.bass import ts
from concourse.bass_utils import run_bass_kernel_spmd

N = 8192
D = 512
NC = 8
M_LOC = N // NC          # 1024 image rows per core
MT = M_LOC // 128        # 8 m-tiles of 128 rows
NT = N // 512            # 16 n-tiles of 512 text cols
KC = D // 128            # 4 contraction chunks

F32 = mybir.dt.float32
BF16 = mybir.dt.bfloat16

# Matmul input dtype: "f32" (exact) or "bf16" (4x PE throughput, ~1e-5 loss err)
MM_DTYPE = "bf16"
# Single matmul streaming 1024 bf16 columns (2 PSUM banks) vs two 512-col MMs
WIDE_MM = False

_CACHE = {}
LAST_RESULTS = None


def _build(scale: float, shift: float, mm_dtype: str, dims=None):
    n, m_loc, kc_n = (N, M_LOC, KC) if dims is None else dims
    mt_n, nt_n = m_loc // 128, n // 1024
    n_mm = 1024 if WIDE_MM else 512
    mmdt = F32 if mm_dtype == "f32" else BF16
    nc = bacc.Bacc("TRN2", debug=False)

    at_d = nc.dram_tensor("at_in", [128, kc_n, m_loc], mmdt, kind="ExternalInput").ap()
    bt_d = nc.dram_tensor("bt_in", [nt_n, 128, kc_n, 1024], mmdt, kind="ExternalInput").ap()
    eye_d = nc.dram_tensor("eye_in", [128, 128], F32, kind="ExternalInput").ap()

    rowsum_d = nc.dram_tensor("rowsum_out", [128, mt_n], F32, kind="ExternalOutput").ap()
    colsum_d = nc.dram_tensor("colsum_out", [1, n], F32, kind="ExternalOutput").ap()
    diag_d = nc.dram_tensor("diag_out", [128, mt_n], F32, kind="ExternalOutput").ap()

    with ExitStack() as ctx:
        tc = ctx.enter_context(tile.TileContext(nc))
        singles = ctx.enter_context(tc.tile_pool(name="singles", bufs=1))
        btp = ctx.enter_context(tc.tile_pool(name="btp", bufs=nt_n))
        expp = ctx.enter_context(tc.tile_pool(name="expp", bufs=6))
        scr = ctx.enter_context(tc.tile_pool(name="scr", bufs=2))
        psum = ctx.enter_context(tc.tile_pool(name="psum", bufs=3, space="PSUM"))
        cacc = ctx.enter_context(tc.tile_pool(name="cacc", bufs=2, space="PSUM"))

        at_t = singles.tile([128, kc_n, m_loc], mmdt)
        bt_tiles = [
            btp.tile([128, kc_n, 1024], mmdt, name=f"bt{nt}", tag="bt")
            for nt in range(nt_n)
        ]
        # Per-chunk loads for the first tiles so the first matmul group can
        # start as soon as its (at, bt0) K-chunks land, not after 2 MB.
        for kc in range(kc_n):
            nc.sync.dma_start(at_t[:, kc, :], at_d[:, kc, :])
            nc.sync.dma_start(bt_tiles[0][:, kc, :], bt_d[0, :, kc, :])
        eye_t = singles.tile([128, 128], F32)
        nc.sync.dma_start(eye_t, eye_d)
        ones_t = singles.tile([128, 1], mmdt)
        nc.vector.memset(ones_t, 1.0)
        bias_t = singles.tile([128, 1], F32)
        nc.vector.memset(bias_t, -shift)

        rowpart = singles.tile([128, mt_n, nt_n], F32)
        rowsum_sb = singles.tile([128, mt_n], F32)
        diag_sb = singles.tile([128, mt_n], F32)

        for nt in range(1, nt_n):
            nc.sync.dma_start(bt_tiles[nt], bt_d[nt])

        # flush_colsum(nt): partition-reduce the finished colacc via two
        # ones-matmuls, copy to SBUF (DVE), DMA out. Deferred one nt so the
        # PE doesn't stall waiting for the DVE accumulate chain.
        def flush_colsum(nt, colacc_sb):
            for h in range(2):
                colacc_ps = cacc.tile([1, 512], F32, name=f"cps{nt}_{h}", tag="cps")
                nc.tensor.matmul(
                    colacc_ps, ones_t, colacc_sb[:, ts(h, 512)], start=True, stop=True
                )
                col_sb = scr.tile([1, 512], F32, name=f"colsb{nt}_{h}", tag="colsb")
                nc.vector.tensor_copy(col_sb, colacc_ps)
                nc.sync.dma_start(colsum_d[:, ts(nt * 2 + h, 512)], col_sb)

        pending = None
        for nt in range(nt_n):
            colacc_sb = scr.tile([128, 1024], mmdt, name=f"cacc{nt}", tag="caccsb", bufs=2)
            for mt in range(mt_n):
                s_ps = psum.tile([128, 1024], F32, name=f"s{nt}_{mt}", tag="spsum")
                for kc in range(kc_n):
                    if WIDE_MM:
                        nc.tensor.matmul(
                            s_ps,
                            at_t[:, kc, ts(mt, 128)],
                            bt_tiles[nt][:, kc, :],
                            start=(kc == 0),
                            stop=(kc == kc_n - 1),
                        )
                    else:
                        for h in range(2):
                            nc.tensor.matmul(
                                s_ps[:, ts(h, 512)],
                                at_t[:, kc, ts(mt, 128)],
                                bt_tiles[nt][:, kc, ts(h, 512)],
                                start=(kc == 0),
                                stop=(kc == kc_n - 1),
                            )
                if pending is not None:
                    flush_colsum(*pending)
                    pending = None
                if nt == (mt * 128) // 1024:
                    # this tile holds the local diagonal block for mt
                    o = (mt * 128) % 1024
                    dscr = scr.tile([128, 128], F32, name=f"dscr{mt}", tag="dscr")
                    nc.vector.tensor_mul(dscr, s_ps[:, o : o + 128], eye_t)
                    nc.vector.tensor_reduce(
                        out=diag_sb[:, mt : mt + 1],
                        in_=dscr,
                        axis=mybir.AxisListType.X,
                        op=mybir.AluOpType.add,
                    )
                e_t = expp.tile([128, 1024], mmdt, name=f"e{nt}_{mt}", tag="exp")
                nc.scalar.activation(
                    e_t,
                    s_ps,
                    mybir.ActivationFunctionType.Exp,
                    bias=bias_t,
                    scale=scale,
                    accum_out=rowpart[:, mt, nt : nt + 1],
                )
                if mt == 0:
                    nc.vector.tensor_copy(colacc_sb, e_t)
                else:
                    nc.vector.tensor_add(colacc_sb, colacc_sb, e_t)
            pending = (nt, colacc_sb)
        flush_colsum(*pending)

        for mt in range(mt_n):
            nc.vector.tensor_reduce(
                out=rowsum_sb[:, mt : mt + 1],
                in_=rowpart[:, mt, :],
                axis=mybir.AxisListType.X,
                op=mybir.AluOpType.add,
            )
        nc.sync.dma_start(rowsum_d, rowsum_sb)
        nc.sync.dma_start(diag_d, diag_sb)

    nc.compile()
    return nc


def _prep_inputs(img, txt, scale, mm_dtype):
    np_mmdt = np.float32 if mm_dtype == "f32" else np.dtype("bfloat16")
    try:
        np.dtype(np_mmdt)
    except TypeError:  # numpy without native bf16: use ml_dtypes
        pass
    if mm_dtype != "f32":
        import ml_dtypes

        np_mmdt = ml_dtypes.bfloat16

    eye = (scale * np.eye(128)).astype(np.float32)
    in_maps = []
    for c in range(NC):
        A = img[c * M_LOC : (c + 1) * M_LOC]                    # [1024, 512]
        at = np.ascontiguousarray(
            A.T.reshape(KC, 128, M_LOC).transpose(1, 0, 2)
        ).astype(np_mmdt)                                       # [128, 4, 1024]
        tr = np.roll(txt, -c * M_LOC, axis=0)                   # local col j -> global (j + c*1024) % N
        bt = np.ascontiguousarray(
            tr.T.reshape(KC, 128, N // 1024, 1024).transpose(2, 1, 0, 3)
        ).astype(np_mmdt)                                       # [8, 128, 4, 1024]
        in_maps.append({"at_in": at, "bt_in": bt, "eye_in": eye})
    return in_maps


def kernel(image_features, text_features, logit_scale):
    global LAST_RESULTS
    img = np.ascontiguousarray(np.asarray(image_features, dtype=np.float32))
    txt = np.ascontiguousarray(np.asarray(text_features, dtype=np.float32))
    scale = float(np.asarray(logit_scale))
    shift = 0.5 * scale

    key = (scale, MM_DTYPE)
    if key not in _CACHE:
        _CACHE[key] = _build(scale, shift, MM_DTYPE)
    nc = _CACHE[key]

    in_maps = _prep_inputs(img, txt, scale, MM_DTYPE)
    res = run_bass_kernel_spmd(nc, in_maps, core_ids=list(range(NC)))
    LAST_RESULTS = res

    colsum_tot = np.zeros(N, dtype=np.float64)
    lse_rows = []
    diags = []
    for c, r in enumerate(res.results):
        rowsum = r["rowsum_out"].astype(np.float64)             # [128, MT] @ [p, mt]
        lse_rows.append(shift + np.log(rowsum.T.reshape(-1)))   # row = mt*128 + p
        diags.append(r["diag_out"].astype(np.float64).T.reshape(-1))
        colsum_tot += np.roll(r["colsum_out"].astype(np.float64).reshape(-1), c * M_LOC)
    lse_row = np.concatenate(lse_rows)
    diag = np.concatenate(diags)
    lse_col = shift + np.log(colsum_tot)

    loss = 0.5 * (np.mean(lse_row - diag) + np.mean(lse_col - diag))
    return np.float32(loss)


# revision 17
# speedup vs baseline: 1.0061x; 1.0061x over previous
"""CLIP contrastive loss on 8 Trainium2 NeuronCores (Bass/Tile).

Strategy (data-parallel over image rows, hint's local_loss path):
  - Core c holds image rows [c*1024, (c+1)*1024) and the FULL text matrix.
  - Text rows are rolled by c*1024 on the host so every core's diagonal
    block sits at local column 0 (the compiled program is core-independent).
  - On device, each core computes its 1024 x 8192 logits block in
    128x512 PSUM tiles (4 accumulating K=128 matmuls each), then:
      * ACT: exp(scale*s - shift) PSUM->SBUF, accum_out = per-row sums
      * PE:  ones[128,1]^T @ exp_tile accumulated in PSUM = column sums
      * DVE: tensor_tensor_reduce against scale*I extracts the diagonal
  - Host: combines per-core row/col exp-sums and diagonals in float64:
      lse = shift + log(sum); loss = mean over both directions.

Fixed-shift logsumexp is numerically safe: logits = scale*cos(theta) are
bounded by +-scale, and shift = scale/2 keeps every term that matters in
normal f32 range (terms below exp(-87) are negligible vs the row max).
"""

from contextlib import ExitStack

import numpy as np

import concourse.bass as bass
from concourse import bacc
import concourse.tile as tile
from concourse import mybir
from concourse.bass import ts
from concourse.bass_utils import run_bass_kernel_spmd

N = 8192
D = 512
NC = 8
M_LOC = N // NC          # 1024 image rows per core
MT = M_LOC // 128        # 8 m-tiles of 128 rows
NT = N // 512            # 16 n-tiles of 512 text cols
KC = D // 128            # 4 contraction chunks

F32 = mybir.dt.float32
BF16 = mybir.dt.bfloat16

# Matmul input dtype: "f32" (exact) or "bf16" (4x PE throughput, ~1e-5 loss err)
MM_DTYPE = "bf16"
# Single matmul streaming 1024 bf16 columns (2 PSUM banks) vs two 512-col MMs
WIDE_MM = False

_CACHE = {}
LAST_RESULTS = None


def _build(scale: float, shift: float, mm_dtype: str, dims=None):
    n, m_loc, kc_n = (N, M_LOC, KC) if dims is None else dims
    mt_n, nt_n = m_loc // 128, n // 1024
    mmdt = F32 if mm_dtype == "f32" else BF16
    nc = bacc.Bacc("TRN2", debug=False)

    at_d = nc.dram_tensor("at_in", [128, kc_n, m_loc], mmdt, kind="ExternalInput").ap()
    bt_d = nc.dram_tensor("bt_in", [nt_n, 128, kc_n, 1024], mmdt, kind="ExternalInput").ap()
    eye_d = nc.dram_tensor("eye_in", [128, 128], F32, kind="ExternalInput").ap()

    rowsum_d = nc.dram_tensor("rowsum_out", [128, mt_n], F32, kind="ExternalOutput").ap()
    colsum_d = nc.dram_tensor("colsum_out", [1, n], F32, kind="ExternalOutput").ap()
    diag_d = nc.dram_tensor("diag_out", [128, mt_n], F32, kind="ExternalOutput").ap()

    with ExitStack() as ctx:
        tc = ctx.enter_context(tile.TileContext(nc))
        singles = ctx.enter_context(tc.tile_pool(name="singles", bufs=1))
        btp = ctx.enter_context(tc.tile_pool(name="btp", bufs=nt_n))
        expp = ctx.enter_context(tc.tile_pool(name="expp", bufs=8))
        scr = ctx.enter_context(tc.tile_pool(name="scr", bufs=2))
        psum = ctx.enter_context(tc.tile_pool(name="psum", bufs=3, space="PSUM"))
        cacc = ctx.enter_context(tc.tile_pool(name="cacc", bufs=2, space="PSUM"))

        at_t = singles.tile([128, kc_n, m_loc], mmdt)
        bt_tiles = [
            btp.tile([128, kc_n, 1024], mmdt, name=f"bt{nt}", tag="bt")
            for nt in range(nt_n)
        ]
        # Per-chunk loads for the first tiles so the first matmul group can
        # start as soon as its (at, bt0) K-chunks land, not after 2 MB.
        for kc in range(kc_n):
            nc.sync.dma_start(at_t[:, kc, :], at_d[:, kc, :])
            nc.sync.dma_start(bt_tiles[0][:, kc, :], bt_d[0, :, kc, :])
        eye_t = singles.tile([128, 128], F32)
        nc.sync.dma_start(eye_t, eye_d)
        ones_t = singles.tile([128, 1], mmdt)
        nc.vector.memset(ones_t, 1.0)
        bias_t = singles.tile([128, 1], F32)
        nc.vector.memset(bias_t, -shift)

        rowpart = singles.tile([128, mt_n, nt_n], F32)
        rowsum_sb = singles.tile([128, mt_n], F32)
        diag_sb = singles.tile([128, mt_n], F32)

        for nt in range(1, nt_n):
            nc.sync.dma_start(bt_tiles[nt], bt_d[nt])

        # flush_colsum(nt): partition-reduce the finished colacc via two
        # ones-matmuls, copy to SBUF (DVE), DMA out. Deferred one nt so the
        # PE doesn't stall waiting for the DVE accumulate chain.
        def flush_colsum(nt, colacc_sb):
            for h in range(2):
                colacc_ps = cacc.tile([1, 512], F32, name=f"cps{nt}_{h}", tag="cps")
                nc.tensor.matmul(
                    colacc_ps, ones_t, colacc_sb[:, ts(h, 512)], start=True, stop=True
                )
                col_sb = scr.tile([1, 512], F32, name=f"colsb{nt}_{h}", tag="colsb")
                nc.vector.tensor_copy(col_sb, colacc_ps)
                nc.sync.dma_start(colsum_d[:, ts(nt * 2 + h, 512)], col_sb)

        pending = None
        for nt in range(nt_n):
            colacc_sb = scr.tile([128, 1024], mmdt, name=f"cacc{nt}", tag="caccsb", bufs=2)
            for mt in range(mt_n):
                s_ps = psum.tile([128, 1024], F32, name=f"s{nt}_{mt}", tag="spsum")
                for kc in range(kc_n):
                    if WIDE_MM:
                        nc.tensor.matmul(
                            s_ps,
                            at_t[:, kc, ts(mt, 128)],
                            bt_tiles[nt][:, kc, :],
                            start=(kc == 0),
                            stop=(kc == kc_n - 1),
                        )
                    else:
                        for h in range(2):
                            nc.tensor.matmul(
                                s_ps[:, ts(h, 512)],
                                at_t[:, kc, ts(mt, 128)],
                                bt_tiles[nt][:, kc, ts(h, 512)],
                                start=(kc == 0),
                                stop=(kc == kc_n - 1),
                            )
                if pending is not None:
                    flush_colsum(*pending)
                    pending = None
                if nt == (mt * 128) // 1024:
                    # this tile holds the local diagonal block for mt
                    o = (mt * 128) % 1024
                    dscr = scr.tile([128, 128], F32, name=f"dscr{mt}", tag="dscr")
                    nc.vector.tensor_mul(dscr, s_ps[:, o : o + 128], eye_t)
                    nc.vector.tensor_reduce(
                        out=diag_sb[:, mt : mt + 1],
                        in_=dscr,
                        axis=mybir.AxisListType.X,
                        op=mybir.AluOpType.add,
                    )
                e_t = expp.tile([128, 1024], mmdt, name=f"e{nt}_{mt}", tag="exp")
                nc.scalar.activation(
                    e_t,
                    s_ps,
                    mybir.ActivationFunctionType.Exp,
                    bias=bias_t,
                    scale=scale,
                    accum_out=rowpart[:, mt, nt : nt + 1],
                )
                if mt == 0:
                    nc.vector.tensor_copy(colacc_sb, e_t)
                else:
                    nc.vector.tensor_add(colacc_sb, colacc_sb, e_t)
            pending = (nt, colacc_sb)
        flush_colsum(*pending)

        for mt in range(mt_n):
            nc.vector.tensor_reduce(
                out=rowsum_sb[:, mt : mt + 1],
                in_=rowpart[:, mt, :],
                axis=mybir.AxisListType.X,
                op=mybir.AluOpType.add,
            )
        nc.sync.dma_start(rowsum_d, rowsum_sb)
        nc.sync.dma_start(diag_d, diag_sb)

    nc.compile()
    return nc


def _prep_inputs(img, txt, scale, mm_dtype):
    np_mmdt = np.float32 if mm_dtype == "f32" else np.dtype("bfloat16")
    try:
        np.dtype(np_mmdt)
    except TypeError:  # numpy without native bf16: use ml_dtypes
        pass
    if mm_dtype != "f32":
        import ml_dtypes

        np_mmdt = ml_dtypes.bfloat16

    eye = (scale * np.eye(128)).astype(np.float32)
    in_maps = []
    for c in range(NC):
        A = img[c * M_LOC : (c + 1) * M_LOC]                    # [1024, 512]
        at = np.ascontiguousarray(
            A.T.reshape(KC, 128, M_LOC).transpose(1, 0, 2)
        ).astype(np_mmdt)                                       # [128, 4, 1024]
        tr = np.roll(txt, -c * M_LOC, axis=0)                   # local col j -> global (j + c*1024) % N
        bt = np.ascontiguousarray(
            tr.T.reshape(KC, 128, N // 1024, 1024).transpose(2, 1, 0, 3)
        ).astype(np_mmdt)                                       # [8, 128, 4, 1024]
        in_maps.append({"at_in": at, "bt_in": bt, "eye_in": eye})
    return in_maps


def kernel(image_features, text_features, logit_scale):
    global LAST_RESULTS
    img = np.ascontiguousarray(np.asarray(image_features, dtype=np.float32))
    txt = np.ascontiguousarray(np.asarray(text_features, dtype=np.float32))
    scale = float(np.asarray(logit_scale))
    shift = 0.5 * scale

    key = (scale, MM_DTYPE)
    if key not in _CACHE:
        _CACHE[key] = _build(scale, shift, MM_DTYPE)
    nc = _CACHE[key]

    in_maps = _prep_inputs(img, txt, scale, MM_DTYPE)
    res = run_bass_kernel_spmd(nc, in_maps, core_ids=list(range(NC)))
    LAST_RESULTS = res

    colsum_tot = np.zeros(N, dtype=np.float64)
    lse_rows = []
    diags = []
    for c, r in enumerate(res.results):
        rowsum = r["rowsum_out"].astype(np.float64)             # [128, MT] @ [p, mt]
        lse_rows.append(shift + np.log(rowsum.T.reshape(-1)))   # row = mt*128 + p
        diags.append(r["diag_out"].astype(np.float64).T.reshape(-1))
        colsum_tot += np.roll(r["colsum_out"].astype(np.float64).reshape(-1), c * M_LOC)
    lse_row = np.concatenate(lse_rows)
    diag = np.concatenate(diags)
    lse_col = shift + np.log(colsum_tot)

    loss = 0.5 * (np.mean(lse_row - diag) + np.mean(lse_col - diag))
    return np.float32(loss)


# revision 19
# speedup vs baseline: 1.0400x; 1.0337x over previous
"""CLIP contrastive loss on 8 Trainium2 NeuronCores (Bass/Tile).

Strategy (data-parallel over image rows, hint's local_loss path):
  - Core c holds image rows [c*1024, (c+1)*1024) and the FULL text matrix.
  - Text rows are rolled by c*1024 on the host so every core's diagonal
    block sits at local column 0 (the compiled program is core-independent).
  - On device, each core computes its 1024 x 8192 logits block in
    128x1024 wide PSUM tiles (two 512-col halves, 4 accumulating K=128
    bf16 matmuls each), then:
      * ACT: one exp(scale*s - shift) per wide tile PSUM->SBUF (bf16),
        accum_out = per-row sums (free with the exp pass)
      * DVE: adds exp tiles into a per-nt column accumulator; per-mt
        diagonal extracted with tensor_mul against scale*I + reduce
      * PE:  two ones[128,1]^T matmuls partition-reduce each finished
        column accumulator (deferred one nt to avoid PE stalls)
  - Host: combines per-core row/col exp-sums and diagonals in float64:
      lse = shift + log(sum); loss = mean over both directions.

Fixed-shift logsumexp is numerically safe: logits = scale*cos(theta) are
bounded by +-scale, and shift = scale/2 keeps every term that matters in
normal f32 range (terms below exp(-87) are negligible vs the row max).
"""

from contextlib import ExitStack

import numpy as np

import concourse.bass as bass
from concourse import bacc
import concourse.tile as tile
from concourse import mybir
from concourse.bass import ts
from concourse.bass_utils import run_bass_kernel_spmd

N = 8192
D = 512
NC = 8
M_LOC = N // NC          # 1024 image rows per core
MT = M_LOC // 128        # 8 m-tiles of 128 rows
NT = N // 512            # 16 n-tiles of 512 text cols
KC = D // 128            # 4 contraction chunks

F32 = mybir.dt.float32
BF16 = mybir.dt.bfloat16

# Matmul input dtype: "f32" (exact) or "bf16" (4x PE throughput, ~1e-5 loss err)
MM_DTYPE = "bf16"
# Single matmul streaming 1024 bf16 columns (2 PSUM banks) vs two 512-col MMs
WIDE_MM = False

_CACHE = {}
LAST_RESULTS = None


def _build(scale: float, shift: float, mm_dtype: str, dims=None):
    n, m_loc, kc_n = (N, M_LOC, KC) if dims is None else dims
    mt_n, nt_n = m_loc // 128, n // 1024
    mmdt = F32 if mm_dtype == "f32" else BF16
    nc = bacc.Bacc("TRN2", debug=False)

    at_d = nc.dram_tensor("at_in", [128, kc_n, m_loc], mmdt, kind="ExternalInput").ap()
    bt_d = nc.dram_tensor("bt_in", [nt_n, 128, kc_n, 1024], mmdt, kind="ExternalInput").ap()
    eye_d = nc.dram_tensor("eye_in", [128, 128], F32, kind="ExternalInput").ap()

    rowsum_d = nc.dram_tensor("rowsum_out", [128, mt_n], F32, kind="ExternalOutput").ap()
    colsum_d = nc.dram_tensor("colsum_out", [nt_n, 128, 1024], mmdt, kind="ExternalOutput").ap()
    diag_d = nc.dram_tensor("diag_out", [128, mt_n], F32, kind="ExternalOutput").ap()

    with ExitStack() as ctx:
        tc = ctx.enter_context(tile.TileContext(nc))
        singles = ctx.enter_context(tc.tile_pool(name="singles", bufs=1))
        btp = ctx.enter_context(tc.tile_pool(name="btp", bufs=nt_n))
        expp = ctx.enter_context(tc.tile_pool(name="expp", bufs=8))
        scr = ctx.enter_context(tc.tile_pool(name="scr", bufs=2))
        psum = ctx.enter_context(tc.tile_pool(name="psum", bufs=3, space="PSUM"))

        at_t = singles.tile([128, kc_n, m_loc], mmdt)
        bt_tiles = [
            btp.tile([128, kc_n, 1024], mmdt, name=f"bt{nt}", tag="bt")
            for nt in range(nt_n)
        ]
        # Per-chunk loads for the first tiles so the first matmul group can
        # start as soon as its (at, bt0) K-chunks land, not after 2 MB.
        for kc in range(kc_n):
            nc.sync.dma_start(at_t[:, kc, :], at_d[:, kc, :])
            nc.sync.dma_start(bt_tiles[0][:, kc, :], bt_d[0, :, kc, :])
        eye_t = singles.tile([128, 128], F32)
        nc.sync.dma_start(eye_t, eye_d)
        bias_t = singles.tile([128, 1], F32)
        nc.vector.memset(bias_t, -shift)

        rowpart = singles.tile([128, mt_n, nt_n], F32)
        rowsum_sb = singles.tile([128, mt_n], F32)
        diag_sb = singles.tile([128, mt_n], F32)

        for nt in range(1, nt_n):
            nc.sync.dma_start(bt_tiles[nt], bt_d[nt])

        for nt in range(nt_n):
            colacc_sb = scr.tile([128, 1024], mmdt, name=f"cacc{nt}", tag="caccsb", bufs=2)
            for mt in range(mt_n):
                s_ps = psum.tile([128, 1024], F32, name=f"s{nt}_{mt}", tag="spsum")
                for kc in range(kc_n):
                    if WIDE_MM:
                        nc.tensor.matmul(
                            s_ps,
                            at_t[:, kc, ts(mt, 128)],
                            bt_tiles[nt][:, kc, :],
                            start=(kc == 0),
                            stop=(kc == kc_n - 1),
                        )
                    else:
                        for h in range(2):
                            nc.tensor.matmul(
                                s_ps[:, ts(h, 512)],
                                at_t[:, kc, ts(mt, 128)],
                                bt_tiles[nt][:, kc, ts(h, 512)],
                                start=(kc == 0),
                                stop=(kc == kc_n - 1),
                            )
                if nt == (mt * 128) // 1024:
                    # this tile holds the local diagonal block for mt
                    o = (mt * 128) % 1024
                    dscr = scr.tile([128, 128], F32, name=f"dscr{mt}", tag="dscr")
                    nc.vector.tensor_mul(dscr, s_ps[:, o : o + 128], eye_t)
                    nc.vector.tensor_reduce(
                        out=diag_sb[:, mt : mt + 1],
                        in_=dscr,
                        axis=mybir.AxisListType.X,
                        op=mybir.AluOpType.add,
                    )
                e_t = expp.tile([128, 1024], mmdt, name=f"e{nt}_{mt}", tag="exp")
                nc.scalar.activation(
                    e_t,
                    s_ps,
                    mybir.ActivationFunctionType.Exp,
                    bias=bias_t,
                    scale=scale,
                    accum_out=rowpart[:, mt, nt : nt + 1],
                )
                if mt == 0:
                    nc.vector.tensor_copy(colacc_sb, e_t)
                else:
                    nc.vector.tensor_add(colacc_sb, colacc_sb, e_t)
            nc.sync.dma_start(colsum_d[nt], colacc_sb)

        for mt in range(mt_n):
            nc.vector.tensor_reduce(
                out=rowsum_sb[:, mt : mt + 1],
                in_=rowpart[:, mt, :],
                axis=mybir.AxisListType.X,
                op=mybir.AluOpType.add,
            )
        nc.sync.dma_start(rowsum_d, rowsum_sb)
        nc.sync.dma_start(diag_d, diag_sb)

    nc.compile()
    return nc


def _prep_inputs(img, txt, scale, mm_dtype):
    np_mmdt = np.float32 if mm_dtype == "f32" else np.dtype("bfloat16")
    try:
        np.dtype(np_mmdt)
    except TypeError:  # numpy without native bf16: use ml_dtypes
        pass
    if mm_dtype != "f32":
        import ml_dtypes

        np_mmdt = ml_dtypes.bfloat16

    eye = (scale * np.eye(128)).astype(np.float32)
    in_maps = []
    for c in range(NC):
        A = img[c * M_LOC : (c + 1) * M_LOC]                    # [1024, 512]
        at = np.ascontiguousarray(
            A.T.reshape(KC, 128, M_LOC).transpose(1, 0, 2)
        ).astype(np_mmdt)                                       # [128, 4, 1024]
        tr = np.roll(txt, -c * M_LOC, axis=0)                   # local col j -> global (j + c*1024) % N
        bt = np.ascontiguousarray(
            tr.T.reshape(KC, 128, N // 1024, 1024).transpose(2, 1, 0, 3)
        ).astype(np_mmdt)                                       # [8, 128, 4, 1024]
        in_maps.append({"at_in": at, "bt_in": bt, "eye_in": eye})
    return in_maps


def kernel(image_features, text_features, logit_scale):
    global LAST_RESULTS
    img = np.ascontiguousarray(np.asarray(image_features, dtype=np.float32))
    txt = np.ascontiguousarray(np.asarray(text_features, dtype=np.float32))
    scale = float(np.asarray(logit_scale))
    shift = 0.5 * scale

    key = (scale, MM_DTYPE)
    if key not in _CACHE:
        _CACHE[key] = _build(scale, shift, MM_DTYPE)
    nc = _CACHE[key]

    in_maps = _prep_inputs(img, txt, scale, MM_DTYPE)
    res = run_bass_kernel_spmd(nc, in_maps, core_ids=list(range(NC)))
    LAST_RESULTS = res

    colsum_tot = np.zeros(N, dtype=np.float64)
    lse_rows = []
    diags = []
    for c, r in enumerate(res.results):
        rowsum = r["rowsum_out"].astype(np.float64)             # [128, MT] @ [p, mt]
        lse_rows.append(shift + np.log(rowsum.T.reshape(-1)))   # row = mt*128 + p
        diags.append(r["diag_out"].astype(np.float64).T.reshape(-1))
        colsum_tot += np.roll(
            r["colsum_out"].astype(np.float64).sum(axis=1).reshape(-1), c * M_LOC
        )
    lse_row = np.concatenate(lse_rows)
    diag = np.concatenate(diags)
    lse_col = shift + np.log(colsum_tot)

    loss = 0.5 * (np.mean(lse_row - diag) + np.mean(lse_col - diag))
    return np.float32(loss)


# revision 21
# speedup vs baseline: 1.0628x; 1.0220x over previous
"""CLIP contrastive loss on 8 Trainium2 NeuronCores (Bass/Tile).

Strategy (data-parallel over image rows, hint's local_loss path):
  - Core c holds image rows [c*1024, (c+1)*1024) and the FULL text matrix.
  - Text rows are rolled by c*1024 on the host so every core's diagonal
    block sits at local column 0 (the compiled program is core-independent).
  - On device, each core computes its 1024 x 8192 logits block in
    128x1024 wide PSUM tiles (two 512-col halves, 4 accumulating K=128
    bf16 matmuls each), then:
      * ACT: one exp(scale*s - shift) per wide tile PSUM->SBUF (bf16),
        accum_out = per-row sums (free with the exp pass)
      * DVE: adds exp tiles into a per-nt [128,1024] bf16 column
        accumulator (DMA'd out whole); per-mt diagonal extracted with
        tensor_mul against scale*I + reduce
  - Host: partition-reduces the column accumulators and combines
    per-core row/col exp-sums and diagonals in float64:
      lse = shift + log(sum); loss = mean over both directions.

Fixed-shift logsumexp is numerically safe: logits = scale*cos(theta) are
bounded by +-scale, and shift = scale/2 keeps every term that matters in
normal f32 range (terms below exp(-87) are negligible vs the row max).
"""

from contextlib import ExitStack

import numpy as np

import concourse.bass as bass
from concourse import bacc
import concourse.tile as tile
from concourse import mybir
from concourse.bass import ts
from concourse.bass_utils import run_bass_kernel_spmd

N = 8192
D = 512
NC = 8
M_LOC = N // NC          # 1024 image rows per core
MT = M_LOC // 128        # 8 m-tiles of 128 rows
NT = N // 512            # 16 n-tiles of 512 text cols
KC = D // 128            # 4 contraction chunks

F32 = mybir.dt.float32
BF16 = mybir.dt.bfloat16

# Matmul input dtype: "f32" (exact) or "bf16" (4x PE throughput, ~1e-5 loss err)
MM_DTYPE = "bf16"
# Single matmul streaming 1024 bf16 columns (2 PSUM banks) vs two 512-col MMs
WIDE_MM = False

_CACHE = {}
LAST_RESULTS = None


def _build(scale: float, shift: float, mm_dtype: str, dims=None):
    n, m_loc, kc_n = (N, M_LOC, KC) if dims is None else dims
    mt_n, nt_n = m_loc // 128, n // 1024
    mmdt = F32 if mm_dtype == "f32" else BF16
    nc = bacc.Bacc("TRN2", debug=False)

    at_d = nc.dram_tensor("at_in", [128, kc_n, m_loc], mmdt, kind="ExternalInput").ap()
    bt_d = nc.dram_tensor("bt_in", [nt_n, 128, kc_n, 1024], mmdt, kind="ExternalInput").ap()
    eye_d = nc.dram_tensor("eye_in", [128, 128], F32, kind="ExternalInput").ap()

    rowsum_d = nc.dram_tensor("rowsum_out", [128, mt_n], F32, kind="ExternalOutput").ap()
    colsum_d = nc.dram_tensor("colsum_out", [nt_n, 128, 1024], mmdt, kind="ExternalOutput").ap()
    diag_d = nc.dram_tensor("diag_out", [128, mt_n], F32, kind="ExternalOutput").ap()

    with ExitStack() as ctx:
        tc = ctx.enter_context(tile.TileContext(nc))
        singles = ctx.enter_context(tc.tile_pool(name="singles", bufs=1))
        btp = ctx.enter_context(tc.tile_pool(name="btp", bufs=nt_n))
        expp = ctx.enter_context(tc.tile_pool(name="expp", bufs=8))
        scr = ctx.enter_context(tc.tile_pool(name="scr", bufs=2))
        psum = ctx.enter_context(tc.tile_pool(name="psum", bufs=4, space="PSUM"))

        at_t = singles.tile([128, kc_n, m_loc], mmdt)
        bt_tiles = [
            btp.tile([128, kc_n, 1024], mmdt, name=f"bt{nt}", tag="bt")
            for nt in range(nt_n)
        ]
        # Per-chunk loads for the first tiles so the first matmul group can
        # start as soon as its (at, bt0) K-chunks land, not after 2 MB.
        for kc in range(kc_n):
            nc.sync.dma_start(at_t[:, kc, :], at_d[:, kc, :])
            nc.sync.dma_start(bt_tiles[0][:, kc, :], bt_d[0, :, kc, :])
        eye_t = singles.tile([128, 128], F32)
        nc.sync.dma_start(eye_t, eye_d)
        bias_t = singles.tile([128, 1], F32)
        nc.vector.memset(bias_t, -shift)

        rowpart = singles.tile([128, mt_n, nt_n], F32)
        rowsum_sb = singles.tile([128, mt_n], F32)
        diag_sb = singles.tile([128, mt_n], F32)

        for nt in range(1, nt_n):
            nc.sync.dma_start(bt_tiles[nt], bt_d[nt])

        for nt in range(nt_n):
            colacc_sb = scr.tile([128, 1024], mmdt, name=f"cacc{nt}", tag="caccsb", bufs=2)
            for mt in range(mt_n):
                s_ps = psum.tile([128, 1024], F32, name=f"s{nt}_{mt}", tag="spsum")
                for kc in range(kc_n):
                    if WIDE_MM:
                        nc.tensor.matmul(
                            s_ps,
                            at_t[:, kc, ts(mt, 128)],
                            bt_tiles[nt][:, kc, :],
                            start=(kc == 0),
                            stop=(kc == kc_n - 1),
                        )
                    else:
                        for h in range(2):
                            nc.tensor.matmul(
                                s_ps[:, ts(h, 512)],
                                at_t[:, kc, ts(mt, 128)],
                                bt_tiles[nt][:, kc, ts(h, 512)],
                                start=(kc == 0),
                                stop=(kc == kc_n - 1),
                            )
                if nt == (mt * 128) // 1024:
                    # this tile holds the local diagonal block for mt
                    o = (mt * 128) % 1024
                    dscr = scr.tile([128, 128], F32, name=f"dscr{mt}", tag="dscr")
                    nc.vector.tensor_mul(dscr, s_ps[:, o : o + 128], eye_t)
                    nc.vector.tensor_reduce(
                        out=diag_sb[:, mt : mt + 1],
                        in_=dscr,
                        axis=mybir.AxisListType.X,
                        op=mybir.AluOpType.add,
                    )
                e_t = expp.tile([128, 1024], mmdt, name=f"e{nt}_{mt}", tag="exp")
                nc.scalar.activation(
                    e_t,
                    s_ps,
                    mybir.ActivationFunctionType.Exp,
                    bias=bias_t,
                    scale=scale,
                    accum_out=rowpart[:, mt, nt : nt + 1],
                )
                if mt == 0:
                    nc.vector.tensor_copy(colacc_sb, e_t)
                else:
                    nc.vector.tensor_add(colacc_sb, colacc_sb, e_t)
            nc.sync.dma_start(colsum_d[nt], colacc_sb)

        for mt in range(mt_n):
            nc.vector.tensor_reduce(
                out=rowsum_sb[:, mt : mt + 1],
                in_=rowpart[:, mt, :],
                axis=mybir.AxisListType.X,
                op=mybir.AluOpType.add,
            )
        nc.sync.dma_start(rowsum_d, rowsum_sb)
        nc.sync.dma_start(diag_d, diag_sb)

    nc.compile()
    return nc


def _prep_inputs(img, txt, scale, mm_dtype):
    np_mmdt = np.float32 if mm_dtype == "f32" else np.dtype("bfloat16")
    try:
        np.dtype(np_mmdt)
    except TypeError:  # numpy without native bf16: use ml_dtypes
        pass
    if mm_dtype != "f32":
        import ml_dtypes

        np_mmdt = ml_dtypes.bfloat16

    eye = (scale * np.eye(128)).astype(np.float32)
    in_maps = []
    for c in range(NC):
        A = img[c * M_LOC : (c + 1) * M_LOC]                    # [1024, 512]
        at = np.ascontiguousarray(
            A.T.reshape(KC, 128, M_LOC).transpose(1, 0, 2)
        ).astype(np_mmdt)                                       # [128, 4, 1024]
        tr = np.roll(txt, -c * M_LOC, axis=0)                   # local col j -> global (j + c*1024) % N
        bt = np.ascontiguousarray(
            tr.T.reshape(KC, 128, N // 1024, 1024).transpose(2, 1, 0, 3)
        ).astype(np_mmdt)                                       # [8, 128, 4, 1024]
        in_maps.append({"at_in": at, "bt_in": bt, "eye_in": eye})
    return in_maps


def kernel(image_features, text_features, logit_scale):
    global LAST_RESULTS
    img = np.ascontiguousarray(np.asarray(image_features, dtype=np.float32))
    txt = np.ascontiguousarray(np.asarray(text_features, dtype=np.float32))
    scale = float(np.asarray(logit_scale))
    shift = 0.5 * scale

    key = (scale, MM_DTYPE)
    if key not in _CACHE:
        _CACHE[key] = _build(scale, shift, MM_DTYPE)
    nc = _CACHE[key]

    in_maps = _prep_inputs(img, txt, scale, MM_DTYPE)
    res = run_bass_kernel_spmd(nc, in_maps, core_ids=list(range(NC)))
    LAST_RESULTS = res

    colsum_tot = np.zeros(N, dtype=np.float64)
    lse_rows = []
    diags = []
    for c, r in enumerate(res.results):
        rowsum = r["rowsum_out"].astype(np.float64)             # [128, MT] @ [p, mt]
        lse_rows.append(shift + np.log(rowsum.T.reshape(-1)))   # row = mt*128 + p
        diags.append(r["diag_out"].astype(np.float64).T.reshape(-1))
        colsum_tot += np.roll(
            r["colsum_out"].astype(np.float64).sum(axis=1).reshape(-1), c * M_LOC
        )
    lse_row = np.concatenate(lse_rows)
    diag = np.concatenate(diags)
    lse_col = shift + np.log(colsum_tot)

    loss = 0.5 * (np.mean(lse_row - diag) + np.mean(lse_col - diag))
    return np.float32(loss)
